# revision 49
# baseline (speedup 1.0000x reference)
"""Trainium2 Bass kernel for GQA attention (B=2, S=2048, D=2048, H=16, G=4 kv-heads,
DH=128) with interleaved RoPE (base 1e6) and causal mask.

Sharding: one (batch b, kv-group g) pair per NeuronCore -> 8 cores. Each core
computes its 4 q-heads against its single kv-head (Megatron-style column-split
of w_q/w_k/w_v, row-split of w_o) and produces a partial (S, D) output-projection
product in bf16; the host sums the 4 partials per batch and adds bo.

Mixed-precision dataflow (f32 PSUM accumulate everywhere):
  - Q/K/V projections: sequence chunk 0 (rows 0..511) in bf16; chunks 1-3 via
    fp8e4 DoubleRow matmuls (two dmodel k-tiles per pass, 2x PE throughput).
    fp8 weights are prescaled x32 on host (dodges e4m3 subnormals) and the
    PSUM eviction applies 1/32.
  - Interleaved RoPE in transposed layout via DVE stream_shuffle (partition
    pair swap) + sign-folded sin table; no PE permutation matmul.
  - Attention per (head, 512-col sq chunk): scoresT tiles [sk=128, sq<=512]
    bf16 on PE; exp on ScalarE with bias -ln2 (keeps e under fp8e4 max) ->
    off-diagonal kv-tile pairs stored fp8 [128,2,512], diagonal tiles bf16
    (masked via DVE). PV and the ones-row exp-sum matmuls run fp8 DoubleRow
    on pairs, bf16 on diagonal tiles, accumulating in shared PSUM.
  - Softmax denominators: reciprocal_approx_fast (51 ULP), partition
    broadcast on GpSimd, normalize-on-evict via DVE.
  - Output projection bf16, partial streamed to DRAM as bf16.

Early causal rows (chunk 0) only ever touch bf16 q/k/v/e, so fp8 noise is
always attenuated by softmax averaging over >=512 keys.
"""
import sys
import os

if '/opt/trn_rl_repo' not in sys.path:
    sys.path.insert(0, '/opt/trn_rl_repo')

import numpy as np
import ml_dtypes
import math

from contextlib import ExitStack

import concourse.bass as bass
import concourse.mybir as mybir
import concourse.tile as tile
from concourse import bacc
import concourse.bass_utils as bass_utils
from concourse.masks import make_identity

BF = mybir.dt.bfloat16
F32 = mybir.dt.float32
F8 = mybir.dt.float8e4
AF = mybir.ActivationFunctionType
ALU = mybir.AluOpType
DR = mybir.MatmulPerfMode.DoubleRow

B, S, D, H, G = 2, 2048, 2048, 16, 4
DH = 128
HPC = H // G          # q heads per core
KT = D // 128         # dmodel k-tiles
NCH = S // 512        # sq chunks
SCALE = float(1.0 / np.sqrt(DH))
NEGLN2 = float(-np.log(2.0))
WS = 32.0             # host-side fp8 weight prescale
N_CORES = 8
SWAP_MASK = [i ^ 1 for i in range(32)]   # partition pair swap for RoPE

TRACE = False          # set by test harness to capture an NTFF profile
DEBUG = False          # add intermediate-tensor outputs for stage-wise checks
LAST_RESULTS = None    # BassKernelResults of the most recent run (for test.py)

_PROGRAM = None


def _build_program():
    nc = bacc.Bacc("TRN2", target_bir_lowering=False, debug=False,
                   num_devices=N_CORES)

    def din(name, shape, dtype=BF):
        return nc.dram_tensor(name, shape, dtype, kind="ExternalInput").ap()

    # pre-tiled on host to [128, KT*cols] partition-major layout so every
    # input DMA is a plain 2D transfer with multi-KB contiguous runs
    xT8_d = din("xT8", [128, KT * S], F8)
    xTb_d = din("xTb", [128, KT * 512])
    wqb_d = din("wqTb", [128, KT * 512])
    wq8_d = din("wqT8", [128, KT * 512], F8)
    wkb_d = din("wkTb", [128, KT * DH])
    wk8_d = din("wkT8", [128, KT * DH], F8)
    wvb_d = din("wvTb", [128, KT * DH])
    wv8_d = din("wvT8", [128, KT * DH], F8)
    wo_d = din("woT", [512, D])
    cos_d = din("cosT", [DH, S])
    sin_d = din("sinTs", [DH, S])     # sign-folded sin table
    mask_d = din("maskLT", [DH, DH])
    bq_d = din("bq", [DH, HPC], F32)
    bk_d = din("bk", [DH, 1], F32)
    bv_d = din("bv", [DH, 1], F32)
    out_d = nc.dram_tensor("part", [S, D], BF, kind="ExternalOutput").ap()
    if DEBUG:
        dbg_q = nc.dram_tensor("dbg_q", [128, S], BF, kind="ExternalOutput").ap()
        dbg_k = nc.dram_tensor("dbg_k", [128, S], BF, kind="ExternalOutput").ap()
        dbg_v = nc.dram_tensor("dbg_v", [128, KT * DH], BF, kind="ExternalOutput").ap()
        dbg_o0 = nc.dram_tensor("dbg_o0", [128, HPC * 512], BF, kind="ExternalOutput").ap()
        dbg_u0 = nc.dram_tensor("dbg_u0", [128, 512], F32, kind="ExternalOutput").ap()
        dbg_r0 = nc.dram_tensor("dbg_r0", [1, 512], F32, kind="ExternalOutput").ap()
        dbg_rc = nc.dram_tensor("dbg_rc", [1, 512], F32, kind="ExternalOutput").ap()
        dbg_rb1 = nc.dram_tensor("dbg_rb1", [128, 512], F32, kind="ExternalOutput").ap()
        dbg_u1 = nc.dram_tensor("dbg_u1", [128, 512], F32, kind="ExternalOutput").ap()

    with tile.TileContext(nc) as tc, ExitStack() as ctx:
        consts = ctx.enter_context(tc.tile_pool(name="consts", bufs=1))
        # Two global PSUM pools (no phase boundaries): poolM rotates among
        # k-proj/q-proj/vtrans/scores/out-proj psums, poolO among v-proj and
        # the attention out/sum accumulators.
        poolM = ctx.enter_context(tc.tile_pool(name="poolM", bufs=5, space="PSUM"))
        poolO = ctx.enter_context(tc.tile_pool(name="poolO", bufs=3, space="PSUM"))
        rawp = ctx.enter_context(tc.tile_pool(name="rawp", bufs=2))
        tmpp = ctx.enter_context(tc.tile_pool(name="tmpp", bufs=3))
        expp = ctx.enter_context(tc.tile_pool(name="expp", bufs=6))
        expb = ctx.enter_context(tc.tile_pool(name="expb", bufs=6))
        rcp = ctx.enter_context(tc.tile_pool(name="rcp", bufs=3))
        osbp = ctx.enter_context(tc.tile_pool(name="osbp", bufs=2))
        outup = ctx.enter_context(tc.tile_pool(name="outup", bufs=4))
        sumsp = ctx.enter_context(tc.tile_pool(name="sumsp", bufs=3))

        # persistent SBUF tensors
        wqb_sb = consts.tile([128, KT, 512], BF, tag="wqb")
        wq8_sb = consts.tile([128, KT, 512], F8, tag="wq8")
        wkb_sb = consts.tile([128, KT, DH], BF, tag="wkb")
        wk8_sb = consts.tile([128, KT, DH], F8, tag="wk8")
        wvb_sb = consts.tile([128, KT, DH], BF, tag="wvb")
        wv8_sb = consts.tile([128, KT, DH], F8, tag="wv8")
        wo_sb = consts.tile([128, HPC, D], BF, tag="wo")
        mask_sb = consts.tile([128, 128], BF, tag="mask")
        bq_sb = consts.tile([128, HPC], F32, tag="bq")
        bk_sb = consts.tile([128, 1], F32, tag="bk")
        bv_sb = consts.tile([128, 1], F32, tag="bv")
        ones_sb = consts.tile([128, 1], BF, tag="ones")
        # dual-fp8 LDWEIGHTS needs the k-pair dim step to be a multiple of 16,
        # and DoubleRow matmuls must write PSUM starting at partition 0: head 1
        # uses a zero-padded ones column at index 32 so its sums land on
        # partition 32 of a partition-0-based [33, 512] output.
        ones8_sb = consts.tile([128, 2, 16], F8, tag="ones8")
        ones8b_sb = consts.tile([128, 2, 48], F8, tag="ones8b")
        negln2_sb = consts.tile([128, 1], F32, tag="negln2")
        qT_sb = [consts.tile([128, S], BF, tag=f"qT{h}", name=f"qT{h}")
                 for h in range(HPC)]
        kT_sb = consts.tile([128, S], BF, tag="kT")
        v_sb = consts.tile([128, KT, DH], BF, tag="v")
        v8_sb = consts.tile([128, KT, DH], F8, tag="v8")
        outT_sb = [consts.tile([128, HPC, 512], BF, tag=f"outT{c}", name=f"outT{c}")
                   for c in range(NCH)]
        xT8_sb = consts.tile([128, KT, S], F8, tag="xT8")
        xTb_sb = consts.tile([128, KT, 512], BF, tag="xTb")
        cos_sb = consts.tile([128, S], BF, tag="cos")
        sin_sb = consts.tile([128, S], BF, tag="sin")
        ident_sb = consts.tile([128, 128], F32, tag="ident")

        # ---- input DMAs: consumption order, issue spread over 3 engine
        # sequencers (each dma_start costs ~0.6us of descriptor-gen on its
        # triggering sequencer).
        _dma_engines = [nc.sync, nc.gpsimd, nc.scalar]
        _dma_i = [0]

        def dma_in(out, in_):
            eng = _dma_engines[_dma_i[0] % len(_dma_engines)]
            _dma_i[0] += 1
            eng.dma_start(out=out, in_=in_)

        def ld(eng, sb, kk0, kk1, dram, cols):
            eng.dma_start(
                out=sb[:, kk0:kk1, :].rearrange("p t c -> p (t c)"),
                in_=dram[:, kk0 * cols:kk1 * cols])

        # Explicit queue plan. sync+gpsimd carry the PE-critical stream in
        # consumption order; the scalar sequencer (idle until evictions start)
        # takes the tiny tensors and the fp8 weights. ~13.5MB of input means
        # ~35us of HBM time: phase 1 + attention chunk 0 must cover it.
        # head-of-stream minis: exactly what the first K/V c0 matmuls need,
        # so the PE starts ~9us instead of ~16us
        ld(nc.sync, wkb_sb, 0, 2, wkb_d, DH)
        ld(nc.gpsimd, wvb_sb, 0, 2, wvb_d, DH)
        ld(nc.sync, xTb_sb, 0, 1, xTb_d, 512)
        ld(nc.gpsimd, xTb_sb, 1, 3, xTb_d, 512)
        ld(nc.sync, xTb_sb, 3, 4, xTb_d, 512)
        ld(nc.sync, wkb_sb, 2, KT, wkb_d, DH)
        ld(nc.gpsimd, wvb_sb, 2, KT, wvb_d, DH)
        ld(nc.sync, xTb_sb, 4, 10, xTb_d, 512)
        ld(nc.gpsimd, xTb_sb, 10, KT, xTb_d, 512)
        nc.scalar.dma_start(out=bk_sb, in_=bk_d)
        nc.scalar.dma_start(out=bv_sb, in_=bv_d)
        nc.scalar.dma_start(out=bq_sb, in_=bq_d)
        nc.scalar.dma_start(out=mask_sb, in_=mask_d)
        nc.scalar.dma_start(out=cos_sb, in_=cos_d)
        nc.scalar.dma_start(out=sin_sb, in_=sin_d)
        ld(nc.sync, wqb_sb, 0, 8, wqb_d, 512)
        ld(nc.gpsimd, wqb_sb, 8, KT, wqb_d, 512)
        ld(nc.scalar, wk8_sb, 0, KT, wk8_d, DH)
        ld(nc.scalar, wv8_sb, 0, KT, wv8_d, DH)
        for qq in range(4):
            eng = (nc.sync, nc.gpsimd)[qq % 2]
            ld(eng, xT8_sb, qq * 4, qq * 4 + 4, xT8_d, S)
        ld(nc.scalar, wq8_sb, 0, KT, wq8_d, 512)
        for h in range(HPC):
            eng = (nc.sync, nc.gpsimd)[h % 2]
            eng.dma_start(out=wo_sb[:, h, :], in_=wo_d[h * 128:(h + 1) * 128, :])

        # output DMAs round-robin over sync+gpsimd (scalar is ACT-busy)
        _dmo_i = [0]

        def dma_out(out, in_):
            eng = (nc.sync, nc.gpsimd)[_dmo_i[0] % 2]
            _dmo_i[0] += 1
            eng.dma_start(out=out, in_=in_)
        nc.vector.memset(ones_sb, 1.0)
        nc.vector.memset(ones8_sb, 1.0)
        nc.vector.memset(ones8b_sb, 0.0)
        nc.vector.memset(ones8b_sb[:, :, 32:33], 1.0)
        nc.vector.memset(negln2_sb, NEGLN2)
        make_identity(nc, ident_sb)

        def rope_store(raw, dst, bias_ap, c):
            # interleaved RoPE: rot = pair-swap(raw) with sign folded into sin
            sh = tmpp.tile([128, 512], BF, tag="sh", name="sh")
            nc.vector.stream_shuffle(sh, raw, SWAP_MASK)
            t1 = tmpp.tile([128, 512], BF, tag="t1", name="t1")
            nc.vector.tensor_mul(t1, raw, cos_sb[:, c * 512:(c + 1) * 512])
            t2 = tmpp.tile([128, 512], BF, tag="t2", name="t2")
            nc.vector.tensor_mul(t2, sh, sin_sb[:, c * 512:(c + 1) * 512])
            # dst = (t2 + bias) + t1
            nc.vector.scalar_tensor_tensor(dst, t2, bias_ap, t1,
                                           op0=ALU.add, op1=ALU.add)

        def evict_raw(ps, c, dtype=BF, bias=None):
            raw = rawp.tile([128, 512], dtype, tag="raw", name="raw")
            scale = 1.0 if c == 0 else 1.0 / WS
            if bias is None:
                if c == 0:
                    nc.scalar.copy(raw, ps)
                else:
                    nc.scalar.activation(raw, ps, func=AF.Identity, scale=scale)
            else:
                nc.scalar.activation(raw, ps, func=AF.Identity, bias=bias,
                                     scale=scale)
            return raw

        def v_evict(ps, c):
            vraw = evict_raw(ps, c, dtype=F32, bias=bv_sb[:, 0:1])
            for j in range(4):
                t = c * 4 + j
                tp = poolM.tile([128, 128], F32, tag="m512", name="vtps")
                nc.tensor.transpose(tp, vraw[:, j * 128:(j + 1) * 128],
                                    ident_sb)
                nc.vector.tensor_copy(v_sb[:, t, :], tp)
                nc.vector.tensor_copy(v8_sb[:, t, :], tp)

        # ---- phase 1 (all bf16, needs only chunk-0 operands): K/V/Q chunk-0
        # projections + ropes, so the PE ramps while the fp8 slabs stream in.
        ps_k0 = poolM.tile([128, 512], F32, tag="m512", name="kps0")
        ps_v0 = poolO.tile([128, 512], F32, tag="o512", name="vps0")
        for kk in range(KT):
            nc.tensor.matmul(ps_k0, wkb_sb[:, kk, :], xTb_sb[:, kk, :],
                             start=(kk == 0), stop=(kk == KT - 1))
            nc.tensor.matmul(ps_v0, wvb_sb[:, kk, :], xTb_sb[:, kk, :],
                             start=(kk == 0), stop=(kk == KT - 1))
        raw = evict_raw(ps_k0, 0)
        rope_store(raw, kT_sb[:, 0:512], bk_sb[:, 0:1], 0)
        v_evict(ps_v0, 0)
        for h in range(HPC):
            psq = poolM.tile([128, 512], F32, tag="m512", name=f"qc0ps{h}")
            for kk in range(KT):
                nc.tensor.matmul(psq, wqb_sb[:, kk, h * 128:(h + 1) * 128],
                                 xTb_sb[:, kk, :],
                                 start=(kk == 0), stop=(kk == KT - 1))
            raw = evict_raw(psq, 0)
            rope_store(raw, qT_sb[h][:, 0:512], bq_sb[:, h:h + 1], 0)

        def emit_kv_dr():
            # ---- phase 2: chunks 1-3 K/V projections via fp8 DoubleRow
            ps_k = {c: poolM.tile([128, 512], F32, tag="m512", name=f"kps{c}")
                    for c in (1, 2, 3)}
            ps_v = {c: poolO.tile([128, 512], F32, tag="o512", name=f"vps{c}")
                    for c in (1, 2)}
            for kp in range(KT // 2):
                sl = slice(2 * kp, 2 * kp + 2)
                for c in (1, 2, 3):
                    nc.tensor.matmul(ps_k[c], wk8_sb[:, sl, :],
                                     xT8_sb[:, sl, c * 512:(c + 1) * 512],
                                     start=(kp == 0), stop=(kp == KT // 2 - 1),
                                     perf_mode=DR)
                for c in (1, 2):
                    nc.tensor.matmul(ps_v[c], wv8_sb[:, sl, :],
                                     xT8_sb[:, sl, c * 512:(c + 1) * 512],
                                     start=(kp == 0), stop=(kp == KT // 2 - 1),
                                     perf_mode=DR)
            vp3 = poolO.tile([128, 512], F32, tag="o512", name="vps3")
            for kp in range(KT // 2):
                sl = slice(2 * kp, 2 * kp + 2)
                nc.tensor.matmul(vp3, wv8_sb[:, sl, :],
                                 xT8_sb[:, sl, 3 * 512:4 * 512],
                                 start=(kp == 0), stop=(kp == KT // 2 - 1),
                                 perf_mode=DR)
            ps_v[3] = vp3
            for c in (1, 2, 3):
                raw = evict_raw(ps_k[c], c)
                rope_store(raw, kT_sb[:, c * 512:(c + 1) * 512],
                           bk_sb[:, 0:1], c)
            for c in (1, 2, 3):
                v_evict(ps_v[c], c)

        def emit_q_proj(h):
            pss = {c: poolM.tile([128, 512], F32, tag="m512", name=f"qps{c}")
                   for c in (1, 2, 3)}
            for kp in range(KT // 2):
                sl = slice(2 * kp, 2 * kp + 2)
                for c in (1, 2, 3):
                    nc.tensor.matmul(pss[c],
                                     wq8_sb[:, sl, h * 128:(h + 1) * 128],
                                     xT8_sb[:, sl, c * 512:(c + 1) * 512],
                                     start=(kp == 0), stop=(kp == KT // 2 - 1),
                                     perf_mode=DR)
            for c in (1, 2, 3):
                raw = evict_raw(pss[c], c)
                rope_store(raw, qT_sb[h][:, c * 512:(c + 1) * 512],
                           bq_sb[:, h:h + 1], c)

        def emit_c_group(m, np_, ns=None, tail=False):
            if ns is None:
                ns = (2 * np_, 2 * np_ + 1)
            mc, mo = divmod(m, 4)
            pso = {n: poolM.tile([128, 512], F32, tag="m512", name=f"cpsum{n}")
                   for n in ns}
            for h in range(HPC):
                for n in ns:
                    nc.tensor.matmul(pso[n],
                                     outT_sb[mc][:, h, mo * 128:(mo + 1) * 128],
                                     wo_sb[:, h, n * 512:(n + 1) * 512],
                                     start=(h == 0), stop=(h == HPC - 1))
            for g in range(0, len(ns), 2):
                np2 = ns[g:g + 2]
                ob = osbp.tile([128, 2, 512], BF, tag="osb", name="osb")
                # evictions on DVE: keep ScalarE free for the attention exps
                for j, n in enumerate(np2):
                    nc.vector.tensor_copy(ob[:, j, :], pso[n])
                eng = nc.sync if tail else None
                if eng is None:
                    dma_out(out_d[m * 128:(m + 1) * 128,
                                  np2[0] * 512:(np2[-1] + 1) * 512], ob)
                else:
                    eng.dma_start(
                        out=out_d[m * 128:(m + 1) * 128,
                                  np2[0] * 512:(np2[-1] + 1) * 512],
                        in_=ob)

        def emit_b_pair(c, hp, fill_ms=(), tail_ms=None):
            npairs = c            # fp8 DoubleRow units of 2 kv tiles... 2*c pairs
            nt = 4 * c + 4
            hs = (2 * hp, 2 * hp + 1)
            cl, ch_ = c * 512, (c + 1) * 512
            out_ps = {h: poolO.tile([128, 512], F32, tag="o512",
                                    name=f"outps{h}") for h in hs}
            # both heads' exp-sum rows share one PSUM bank (partitions 0/32)
            sums_pair = poolO.tile([33, 512], F32, tag="o512", name="sums_pair")

            # units: 2c fp8 pairs (tiles 2u, 2u+1), then 4 bf16 diagonal tiles
            def emit_scores(u):
                es = {}
                if u < 2 * c:
                    for h in hs:
                        e8 = expp.tile([128, 2, 512], F8, tag="e8", name="e8")
                        for i in (0, 1):
                            t = 2 * u + i
                            s_ps = poolM.tile([128, 512], F32, tag="m512",
                                              name="s_ps")
                            nc.tensor.matmul(s_ps,
                                             kT_sb[:, t * 128:(t + 1) * 128],
                                             qT_sb[h][:, cl:ch_],
                                             start=True, stop=True)
                            nc.scalar.activation(e8[:, i, :], s_ps,
                                                 func=AF.Exp, scale=SCALE,
                                                 bias=negln2_sb[:, 0:1])
                        es[h] = e8
                else:
                    jb = u - 2 * c
                    t = 4 * c + jb
                    off = jb * 128
                    for h in hs:
                        s_ps = poolM.tile([128, 512], F32, tag="m512",
                                          name="s_ps")
                        nc.tensor.matmul(s_ps[:, off:],
                                         kT_sb[:, t * 128:(t + 1) * 128],
                                         qT_sb[h][:, cl + off:ch_],
                                         start=True, stop=True)
                        e = expb.tile([128, 512], BF, tag="exp", name="e")
                        nc.scalar.activation(e[:, off:], s_ps[:, off:],
                                             func=AF.Exp, scale=SCALE,
                                             bias=negln2_sb[:, 0:1])
                        nc.vector.tensor_mul(e[:, off:off + 128],
                                             e[:, off:off + 128], mask_sb)
                        es[h] = e
                return es

            def emit_consume(u, es):
                first = (u == 0)
                if u < 2 * c:
                    sl = slice(2 * u, 2 * u + 2)
                    # head 1 first (33-row write incl. zeroed rows 0-31), so
                    # head 0's start=True re-zeroes only row 0 afterwards
                    nc.tensor.matmul(sums_pair[0:33, :],
                                     ones8b_sb[:, :, 0:33], es[hs[1]],
                                     start=first, stop=False, perf_mode=DR)
                    nc.tensor.matmul(sums_pair[0:1, :],
                                     ones8_sb[:, :, 0:1], es[hs[0]],
                                     start=first, stop=False, perf_mode=DR)
                    for h in hs:
                        nc.tensor.matmul(out_ps[h], v8_sb[:, sl, :], es[h],
                                         start=first, stop=False,
                                         perf_mode=DR)
                else:
                    jb = u - 2 * c
                    t = 4 * c + jb
                    off = jb * 128
                    last = (jb == 3)
                    for i, h in enumerate(hs):
                        nc.tensor.matmul(sums_pair[32 * i:32 * i + 1, off:],
                                         ones_sb, es[h][:, off:],
                                         start=first, stop=last)
                    for h in hs:
                        nc.tensor.matmul(out_ps[h][:, off:], v_sb[:, t, :],
                                         es[h][:, off:],
                                         start=first, stop=last)

            # software pipeline: scores for u+1 issue before the ones/PV
            # consumers of u, so PE never waits on exp
            nu = 2 * c + 4
            prev = None
            for u in range(nu):
                es = emit_scores(u)
                if prev is not None:
                    emit_consume(u - 1, prev)
                prev = es
            emit_consume(nu - 1, prev)

            # evict accumulators to SBUF so the PSUM banks free quickly; the
            # reciprocal/broadcast/normalize chain runs lazily off-PSUM
            # (outT[c] is only needed by out-proj work a chunk boundary later).
            outU = {}
            for h in hs:
                u = outup.tile([128, 512], F32, tag="outU", name="outU")
                nc.vector.tensor_copy(u, out_ps[h])
                outU[h] = u
            # custom-DVE ops ignore the input AP's partition offset (read
            # partition 0): head 1's sums must first move to a partition-0
            # tile via a regular DVE copy (which does handle quadrant shifts)
            recips = {}
            for i, h in enumerate(hs):
                src = sums_pair[32 * i:32 * i + 1, :]
                if i > 0:
                    s1 = sumsp.tile([1, 512], F32, tag="s1", name="s1")
                    nc.vector.tensor_copy(s1, src)
                    src = s1
                recip = rcp.tile([1, 512], F32, tag="recip", name="recip")
                nc.vector.reciprocal_approx_fast(recip, src)
                recips[h] = recip
            if DEBUG and c == 0 and hp == 0:
                nc.sync.dma_start(out=dbg_u0, in_=outU[0])
                nc.sync.dma_start(out=dbg_r0, in_=recips[0])
                nc.sync.dma_start(out=dbg_rc, in_=recips[1])
                nc.sync.dma_start(out=dbg_u1, in_=outU[1])
            rbcs = {}
            for h in hs:
                rbc = rcp.tile([128, 512], F32, tag="rbc", name="rbc")
                nc.gpsimd.partition_broadcast(rbc, recips[h])
                rbcs[h] = rbc
            if DEBUG and c == 0 and hp == 0:
                nc.sync.dma_start(out=dbg_rb1, in_=rbcs[1])
            # fill the boundary chain latency with out-proj work of the
            # previous chunk (its outT rows are complete); on the final pair
            # these PE fills cover the recip/broadcast chain before the
            # slab-normalized tail out-proj below
            for m in fill_ms:
                for np_ in range(NCH // 2):
                    emit_c_group(m, np_)
            if tail_ms is None:
                for h in hs:
                    nc.vector.tensor_mul(outT_sb[c][:, h, :], outU[h], rbcs[h])
            else:
                # final pair: normalize per 128-col slab so each slab's
                # out-proj m-tile starts without waiting for the full chain
                for mo in range(4):
                    sl = slice(mo * 128, (mo + 1) * 128)
                    for h in hs:
                        nc.vector.tensor_mul(outT_sb[c][:, h, sl],
                                             outU[h][:, sl], rbcs[h][:, sl])
                    emit_c_group(tail_ms[mo], 0, ns=(0, 1, 2, 3), tail=True)

        # ---- interleave: attention chunk 0 needs only phase-1 (bf16)
        # results, so it runs while the fp8 slabs finish streaming in; the
        # DoubleRow projections follow.
        emit_b_pair(0, 0)
        emit_b_pair(0, 1)
        emit_kv_dr()
        for h in range(HPC):
            emit_q_proj(h)
        for c in range(1, NCH):
            for hp in range(HPC // 2):
                base = 4 * (c - 1) + 2 * hp
                last = (c == NCH - 1 and hp == HPC // 2 - 1)
                emit_b_pair(c, hp, fill_ms=(base, base + 1),
                            tail_ms=tuple(range(12, 16)) if last else None)
        if DEBUG:
            nc.sync.dma_start(out=dbg_q, in_=qT_sb[0])
            nc.sync.dma_start(out=dbg_k, in_=kT_sb)
            nc.sync.dma_start(out=dbg_v, in_=v_sb)
            nc.sync.dma_start(out=dbg_o0, in_=outT_sb[0])

    nc.compile()
    return nc


def _get_program():
    global _PROGRAM
    if _PROGRAM is None:
        _PROGRAM = _build_program()
    return _PROGRAM


def _host_tables():
    bf16 = ml_dtypes.bfloat16
    pos = np.arange(S, dtype=np.float32)[:, None]
    i = np.arange(DH // 2, dtype=np.float32)
    omega = np.exp((-2.0 * i / DH * np.log(np.float32(1_000_000.0))).astype(np.float32))
    ang = (pos * omega).astype(np.float32)
    sinT = np.ascontiguousarray(np.repeat(np.sin(ang), 2, axis=-1).T)
    cosT = np.ascontiguousarray(np.repeat(np.cos(ang), 2, axis=-1).T)
    # fold the rotate-half sign into the sin table: row 2i -> -sin, 2i+1 -> +sin
    rowsign = np.where(np.arange(DH) % 2 == 0, -1.0, 1.0).astype(np.float32)
    sinTs = sinT * rowsign[:, None]
    maskLT = np.triu(np.ones((128, 128), np.float32)).astype(bf16)
    return cosT.astype(bf16), sinTs.astype(bf16), maskLT


def _install_ntff_hook():
    """Optional: register the axon NTFF profiling hook (missing antenv.axon_hooks
    shim) so run_bass_kernel_spmd(trace=True) can capture HW exec time."""
    import types
    try:
        import antenv
        if 'antenv.axon_hooks' not in sys.modules:
            mod = types.ModuleType('antenv.axon_hooks')
            _hook = [None]
            mod.set_axon_ntff_profile_hook = lambda h: _hook.__setitem__(0, h)
            mod.get_axon_ntff_profile_hook = lambda: _hook[0]
            sys.modules['antenv.axon_hooks'] = mod
            antenv.axon_hooks = mod
        if '/root/.axon_site' not in sys.path:
            sys.path.insert(0, '/root/.axon_site')
        from trn_agent_boot.trn_boot import _ntff_profile_via_ctypes
        sys.modules['antenv.axon_hooks'].set_axon_ntff_profile_hook(
            _ntff_profile_via_ctypes('/opt/axon/libaxon_pjrt.so'))
        bass_utils.upload_artifacts = lambda tmpdir: tmpdir
        return True
    except Exception:
        return False


def kernel(x, wq, bq, wk, bk, wv, bv, wo, bo, masked=None, **_unused):
    global LAST_RESULTS
    bf16 = ml_dtypes.bfloat16
    f8 = ml_dtypes.float8_e4m3
    nc = _get_program()

    x = np.asarray(x, np.float32)
    wq = np.asarray(wq, np.float32)
    wk = np.asarray(wk, np.float32)
    wv = np.asarray(wv, np.float32)
    wo = np.asarray(wo, np.float32)
    bq = np.asarray(bq, np.float32)
    bk = np.asarray(bk, np.float32)
    bv = np.asarray(bv, np.float32)
    bo = np.asarray(bo, np.float32)

    cosT, sinTs, maskLT = _host_tables()

    def tileize(aT, cols):
        # [D, cols] -> [128, KT*cols] partition-major (matches SBUF layout)
        return np.ascontiguousarray(
            aT.reshape(KT, 128, cols).transpose(1, 0, 2).reshape(128, KT * cols))

    xT = [np.ascontiguousarray(x[b].T) for b in range(B)]
    xT8 = [tileize(np.asarray(t, f8), S) for t in xT]
    xTb = [tileize(np.asarray(t[:, :512], bf16), 512) for t in xT]
    in_maps = []
    for core in range(N_CORES):
        b, g = divmod(core, G)
        cs = slice(g * 512, (g + 1) * 512)          # q-channel / out-channel slice
        ks = slice(g * 128, (g + 1) * 128)          # kv-channel slice
        wqT = np.ascontiguousarray(wq[cs, :].T)
        wkT = np.ascontiguousarray(wk[ks, :].T)
        wvT = np.ascontiguousarray(wv[ks, :].T)
        in_maps.append({
            "xT8": xT8[b],
            "xTb": xTb[b],
            "wqTb": tileize(wqT.astype(bf16), 512),
            "wqT8": tileize(np.asarray(wqT * WS, f8), 512),
            "wkTb": tileize(wkT.astype(bf16), DH),
            "wkT8": tileize(np.asarray(wkT * WS, f8), DH),
            "wvTb": tileize(wvT.astype(bf16), DH),
            "wvT8": tileize(np.asarray(wvT * WS, f8), DH),
            "woT": np.ascontiguousarray(wo[:, cs].T).astype(bf16),
            "cosT": cosT,
            "sinTs": sinTs,
            "maskLT": maskLT,
            "bq": np.ascontiguousarray(bq[cs].reshape(HPC, DH).T),
            "bk": np.ascontiguousarray(bk[ks].reshape(DH, 1)),
            "bv": np.ascontiguousarray(bv[ks].reshape(DH, 1)),
        })

    trace = bool(TRACE)
    if trace:
        trace = _install_ntff_hook()
    res = bass_utils.run_bass_kernel_spmd(nc, in_maps,
                                          core_ids=list(range(N_CORES)),
                                          trace=trace)
    LAST_RESULTS = res

    out = np.zeros((B, S, D), np.float32)
    for core in range(N_CORES):
        b = core // G
        out[b] += np.asarray(res.results[core]["part"], np.float32)
    out += bo[None, None, :]
    return out


# revision 50
# speedup vs baseline: 1.1617x; 1.1617x over previous
"""Trainium2 Bass kernel for GQA attention (B=2, S=2048, D=2048, H=16, G=4 kv-heads,
DH=128) with interleaved RoPE (base 1e6) and causal mask.

Sharding: one (batch b, kv-group g) pair per NeuronCore -> 8 cores. Each core
computes its 4 q-heads against its single kv-head (Megatron-style column-split
of w_q/w_k/w_v, row-split of w_o) and produces a partial (S, D) output-projection
product in bf16; the host sums the 4 partials per batch and adds bo.

Mixed-precision dataflow (f32 PSUM accumulate everywhere):
  - Q/K/V projections: sequence chunk 0 (rows 0..511) in bf16; chunks 1-3 via
    fp8e4 DoubleRow matmuls (two dmodel k-tiles per pass, 2x PE throughput).
    fp8 weights are prescaled x32 on host (dodges e4m3 subnormals) and the
    PSUM eviction applies 1/32.
  - Interleaved RoPE in transposed layout via DVE stream_shuffle (partition
    pair swap) + sign-folded sin table; no PE permutation matmul.
  - Attention per (head, 512-col sq chunk): scoresT tiles [sk=128, sq<=512]
    bf16 on PE; exp on ScalarE with bias -ln2 (keeps e under fp8e4 max) ->
    off-diagonal kv-tile pairs stored fp8 [128,2,512], diagonal tiles bf16
    (masked via DVE). PV and the ones-row exp-sum matmuls run fp8 DoubleRow
    on pairs, bf16 on diagonal tiles, accumulating in shared PSUM.
  - Softmax denominators: reciprocal_approx_fast (51 ULP), partition
    broadcast on GpSimd, normalize-on-evict via DVE. (Custom DVE ops ignore
    the input AP partition offset, so head 1's sums first move to a
    partition-0 tile with a regular DVE copy.)
  - Output projection bf16 (errors there hit the final output unattenuated,
    so no fp8), PSUM evictions on DVE to keep ScalarE free for exps,
    partial streamed to DRAM as bf16 in 2-tile slabs.

Early causal rows (chunk 0) only ever touch bf16 q/k/v/e, so fp8 noise is
always attenuated by softmax averaging over >=512 keys.

Schedule: all inputs are host-pre-tiled to [128, KT*cols] so every DMA is a
contiguous 2D transfer (sub-2KB-run patterns measured ~4x slower); phase 1
(bf16 chunk-0 K/V/Q + ropes) starts off ~130KB of head-of-stream mini DMAs
while the fp8 slabs stream in; attention chunk 0 needs only phase-1 results
and covers the rest of the input stream; out-proj work for completed chunks
fills each pair's normalize-chain latency; the final pair normalizes per
128-col slab so each slab immediately unlocks its out-proj m-tile.
"""
import sys
import os

if '/opt/trn_rl_repo' not in sys.path:
    sys.path.insert(0, '/opt/trn_rl_repo')

import numpy as np
import ml_dtypes
import math

from contextlib import ExitStack

import concourse.bass as bass
import concourse.mybir as mybir
import concourse.tile as tile
from concourse import bacc
import concourse.bass_utils as bass_utils
from concourse.masks import make_identity

BF = mybir.dt.bfloat16
F32 = mybir.dt.float32
F8 = mybir.dt.float8e4
AF = mybir.ActivationFunctionType
ALU = mybir.AluOpType
DR = mybir.MatmulPerfMode.DoubleRow

B, S, D, H, G = 2, 2048, 2048, 16, 4
DH = 128
HPC = H // G          # q heads per core
KT = D // 128         # dmodel k-tiles
NCH = S // 512        # sq chunks
SCALE = float(1.0 / np.sqrt(DH))
NEGLN2 = float(-np.log(2.0))
WS = 32.0             # host-side fp8 weight prescale
N_CORES = 8
SWAP_MASK = [i ^ 1 for i in range(32)]   # partition pair swap for RoPE

TRACE = False          # set by test harness to capture an NTFF profile
DEBUG = False          # add intermediate-tensor outputs for stage-wise checks
LAST_RESULTS = None    # BassKernelResults of the most recent run (for test.py)

_PROGRAM = None


def _build_program():
    nc = bacc.Bacc("TRN2", target_bir_lowering=False, debug=False,
                   num_devices=N_CORES)

    def din(name, shape, dtype=BF):
        return nc.dram_tensor(name, shape, dtype, kind="ExternalInput").ap()

    # pre-tiled on host to [128, KT*cols] partition-major layout so every
    # input DMA is a plain 2D transfer with multi-KB contiguous runs
    xT8_d = din("xT8", [128, KT * S], F8)
    xTb_d = din("xTb", [128, KT * 512])
    wqb_d = din("wqTb", [128, KT * 512])
    wq8_d = din("wqT8", [128, KT * 512], F8)
    wkb_d = din("wkTb", [128, KT * DH])
    wk8_d = din("wkT8", [128, KT * DH], F8)
    wvb_d = din("wvTb", [128, KT * DH])
    wv8_d = din("wvT8", [128, KT * DH], F8)
    wo_d = din("woT", [512, D])
    cos_d = din("cosT", [DH, S])
    sin_d = din("sinTs", [DH, S])     # sign-folded sin table
    mask_d = din("maskLT", [DH, DH])
    bq_d = din("bq", [DH, HPC], F32)
    bk_d = din("bk", [DH, 1], F32)
    bv_d = din("bv", [DH, 1], F32)
    out_d = nc.dram_tensor("part", [S, D], BF, kind="ExternalOutput").ap()
    if DEBUG:
        dbg_q = nc.dram_tensor("dbg_q", [128, S], BF, kind="ExternalOutput").ap()
        dbg_k = nc.dram_tensor("dbg_k", [128, S], BF, kind="ExternalOutput").ap()
        dbg_v = nc.dram_tensor("dbg_v", [128, KT * DH], BF, kind="ExternalOutput").ap()
        dbg_o0 = nc.dram_tensor("dbg_o0", [128, HPC * 512], BF, kind="ExternalOutput").ap()
        dbg_u0 = nc.dram_tensor("dbg_u0", [128, 512], F32, kind="ExternalOutput").ap()
        dbg_r0 = nc.dram_tensor("dbg_r0", [1, 512], F32, kind="ExternalOutput").ap()
        dbg_rc = nc.dram_tensor("dbg_rc", [1, 512], F32, kind="ExternalOutput").ap()
        dbg_rb1 = nc.dram_tensor("dbg_rb1", [128, 512], F32, kind="ExternalOutput").ap()
        dbg_u1 = nc.dram_tensor("dbg_u1", [128, 512], F32, kind="ExternalOutput").ap()

    with tile.TileContext(nc) as tc, ExitStack() as ctx:
        consts = ctx.enter_context(tc.tile_pool(name="consts", bufs=1))
        # Two global PSUM pools (no phase boundaries): poolM rotates among
        # k-proj/q-proj/vtrans/scores/out-proj psums, poolO among v-proj and
        # the attention out/sum accumulators.
        poolM = ctx.enter_context(tc.tile_pool(name="poolM", bufs=5, space="PSUM"))
        poolO = ctx.enter_context(tc.tile_pool(name="poolO", bufs=3, space="PSUM"))
        rawp = ctx.enter_context(tc.tile_pool(name="rawp", bufs=2))
        tmpp = ctx.enter_context(tc.tile_pool(name="tmpp", bufs=3))
        expp = ctx.enter_context(tc.tile_pool(name="expp", bufs=6))
        expb = ctx.enter_context(tc.tile_pool(name="expb", bufs=6))
        rcp = ctx.enter_context(tc.tile_pool(name="rcp", bufs=3))
        osbp = ctx.enter_context(tc.tile_pool(name="osbp", bufs=2))
        outup = ctx.enter_context(tc.tile_pool(name="outup", bufs=4))
        sumsp = ctx.enter_context(tc.tile_pool(name="sumsp", bufs=3))

        # persistent SBUF tensors
        wqb_sb = consts.tile([128, KT, 512], BF, tag="wqb")
        wq8_sb = consts.tile([128, KT, 512], F8, tag="wq8")
        wkb_sb = consts.tile([128, KT, DH], BF, tag="wkb")
        wk8_sb = consts.tile([128, KT, DH], F8, tag="wk8")
        wvb_sb = consts.tile([128, KT, DH], BF, tag="wvb")
        wv8_sb = consts.tile([128, KT, DH], F8, tag="wv8")
        wo_sb = consts.tile([128, HPC, D], BF, tag="wo")
        mask_sb = consts.tile([128, 128], BF, tag="mask")
        bq_sb = consts.tile([128, HPC], F32, tag="bq")
        bk_sb = consts.tile([128, 1], F32, tag="bk")
        bv_sb = consts.tile([128, 1], F32, tag="bv")
        ones_sb = consts.tile([128, 1], BF, tag="ones")
        # dual-fp8 LDWEIGHTS needs the k-pair dim step to be a multiple of 16,
        # and DoubleRow matmuls must write PSUM starting at partition 0: head 1
        # uses a zero-padded ones column at index 32 so its sums land on
        # partition 32 of a partition-0-based [33, 512] output.
        ones8_sb = consts.tile([128, 2, 16], F8, tag="ones8")
        ones8b_sb = consts.tile([128, 2, 48], F8, tag="ones8b")
        negln2_sb = consts.tile([128, 1], F32, tag="negln2")
        qT_sb = [consts.tile([128, S], BF, tag=f"qT{h}", name=f"qT{h}")
                 for h in range(HPC)]
        kT_sb = consts.tile([128, S], BF, tag="kT")
        v_sb = consts.tile([128, KT, DH], BF, tag="v")
        v8_sb = consts.tile([128, KT, DH], F8, tag="v8")
        outT_sb = [consts.tile([128, HPC, 512], BF, tag=f"outT{c}", name=f"outT{c}")
                   for c in range(NCH)]
        xT8_sb = consts.tile([128, KT, S], F8, tag="xT8")
        xTb_sb = consts.tile([128, KT, 512], BF, tag="xTb")
        cos_sb = consts.tile([128, S], BF, tag="cos")
        sin_sb = consts.tile([128, S], BF, tag="sin")
        ident_sb = consts.tile([128, 128], F32, tag="ident")

        # ---- input DMAs: consumption order, issue spread over 3 engine
        # sequencers (each dma_start costs ~0.6us of descriptor-gen on its
        # triggering sequencer).
        _dma_engines = [nc.sync, nc.gpsimd, nc.scalar]
        _dma_i = [0]

        def dma_in(out, in_):
            eng = _dma_engines[_dma_i[0] % len(_dma_engines)]
            _dma_i[0] += 1
            eng.dma_start(out=out, in_=in_)

        def ld(eng, sb, kk0, kk1, dram, cols):
            eng.dma_start(
                out=sb[:, kk0:kk1, :].rearrange("p t c -> p (t c)"),
                in_=dram[:, kk0 * cols:kk1 * cols])

        # Explicit queue plan. sync+gpsimd carry the PE-critical stream in
        # consumption order; the scalar sequencer (idle until evictions start)
        # takes the tiny tensors and the fp8 weights. ~13.5MB of input means
        # ~35us of HBM time: phase 1 + attention chunk 0 must cover it.
        # head-of-stream minis: exactly what the first K/V c0 matmuls need,
        # so the PE starts ~9us instead of ~16us
        ld(nc.sync, wkb_sb, 0, 2, wkb_d, DH)
        ld(nc.gpsimd, wvb_sb, 0, 2, wvb_d, DH)
        ld(nc.sync, xTb_sb, 0, 1, xTb_d, 512)
        ld(nc.gpsimd, xTb_sb, 1, 3, xTb_d, 512)
        ld(nc.sync, xTb_sb, 3, 4, xTb_d, 512)
        ld(nc.sync, wkb_sb, 2, KT, wkb_d, DH)
        ld(nc.gpsimd, wvb_sb, 2, KT, wvb_d, DH)
        ld(nc.sync, xTb_sb, 4, 10, xTb_d, 512)
        ld(nc.gpsimd, xTb_sb, 10, KT, xTb_d, 512)
        nc.scalar.dma_start(out=bk_sb, in_=bk_d)
        nc.scalar.dma_start(out=bv_sb, in_=bv_d)
        nc.scalar.dma_start(out=bq_sb, in_=bq_d)
        nc.scalar.dma_start(out=mask_sb, in_=mask_d)
        nc.scalar.dma_start(out=cos_sb, in_=cos_d)
        nc.scalar.dma_start(out=sin_sb, in_=sin_d)
        ld(nc.sync, wqb_sb, 0, 8, wqb_d, 512)
        ld(nc.gpsimd, wqb_sb, 8, KT, wqb_d, 512)
        ld(nc.scalar, wk8_sb, 0, KT, wk8_d, DH)
        ld(nc.scalar, wv8_sb, 0, KT, wv8_d, DH)
        for qq in range(4):
            eng = (nc.sync, nc.gpsimd)[qq % 2]
            ld(eng, xT8_sb, qq * 4, qq * 4 + 4, xT8_d, S)
        ld(nc.scalar, wq8_sb, 0, KT, wq8_d, 512)
        for h in range(HPC):
            eng = (nc.sync, nc.gpsimd)[h % 2]
            eng.dma_start(out=wo_sb[:, h, :], in_=wo_d[h * 128:(h + 1) * 128, :])

        # output DMAs round-robin over sync+gpsimd (scalar is ACT-busy)
        _dmo_i = [0]

        def dma_out(out, in_):
            eng = (nc.sync, nc.gpsimd)[_dmo_i[0] % 2]
            _dmo_i[0] += 1
            eng.dma_start(out=out, in_=in_)
        nc.vector.memset(ones_sb, 1.0)
        nc.vector.memset(ones8_sb, 1.0)
        nc.vector.memset(ones8b_sb, 0.0)
        nc.vector.memset(ones8b_sb[:, :, 32:33], 1.0)
        nc.vector.memset(negln2_sb, NEGLN2)
        make_identity(nc, ident_sb)

        def rope_store(raw, dst, bias_ap, c):
            # interleaved RoPE: rot = pair-swap(raw) with sign folded into sin
            sh = tmpp.tile([128, 512], BF, tag="sh", name="sh")
            nc.vector.stream_shuffle(sh, raw, SWAP_MASK)
            t1 = tmpp.tile([128, 512], BF, tag="t1", name="t1")
            nc.vector.tensor_mul(t1, raw, cos_sb[:, c * 512:(c + 1) * 512])
            t2 = tmpp.tile([128, 512], BF, tag="t2", name="t2")
            nc.vector.tensor_mul(t2, sh, sin_sb[:, c * 512:(c + 1) * 512])
            # dst = (t2 + bias) + t1
            nc.vector.scalar_tensor_tensor(dst, t2, bias_ap, t1,
                                           op0=ALU.add, op1=ALU.add)

        def evict_raw(ps, c, dtype=BF, bias=None):
            raw = rawp.tile([128, 512], dtype, tag="raw", name="raw")
            scale = 1.0 if c == 0 else 1.0 / WS
            if bias is None:
                if c == 0:
                    nc.scalar.copy(raw, ps)
                else:
                    nc.scalar.activation(raw, ps, func=AF.Identity, scale=scale)
            else:
                nc.scalar.activation(raw, ps, func=AF.Identity, bias=bias,
                                     scale=scale)
            return raw

        def v_evict(ps, c):
            vraw = evict_raw(ps, c, dtype=F32, bias=bv_sb[:, 0:1])
            for j in range(4):
                t = c * 4 + j
                tp = poolM.tile([128, 128], F32, tag="m512", name="vtps")
                nc.tensor.transpose(tp, vraw[:, j * 128:(j + 1) * 128],
                                    ident_sb)
                nc.vector.tensor_copy(v_sb[:, t, :], tp)
                nc.vector.tensor_copy(v8_sb[:, t, :], tp)

        # ---- phase 1 (all bf16, needs only chunk-0 operands): K/V/Q chunk-0
        # projections + ropes, so the PE ramps while the fp8 slabs stream in.
        ps_k0 = poolM.tile([128, 512], F32, tag="m512", name="kps0")
        ps_v0 = poolO.tile([128, 512], F32, tag="o512", name="vps0")
        for kk in range(KT):
            nc.tensor.matmul(ps_k0, wkb_sb[:, kk, :], xTb_sb[:, kk, :],
                             start=(kk == 0), stop=(kk == KT - 1))
            nc.tensor.matmul(ps_v0, wvb_sb[:, kk, :], xTb_sb[:, kk, :],
                             start=(kk == 0), stop=(kk == KT - 1))
        raw = evict_raw(ps_k0, 0)
        rope_store(raw, kT_sb[:, 0:512], bk_sb[:, 0:1], 0)
        v_evict(ps_v0, 0)
        for h in range(HPC):
            psq = poolM.tile([128, 512], F32, tag="m512", name=f"qc0ps{h}")
            for kk in range(KT):
                nc.tensor.matmul(psq, wqb_sb[:, kk, h * 128:(h + 1) * 128],
                                 xTb_sb[:, kk, :],
                                 start=(kk == 0), stop=(kk == KT - 1))
            raw = evict_raw(psq, 0)
            rope_store(raw, qT_sb[h][:, 0:512], bq_sb[:, h:h + 1], 0)

        def emit_kv_dr():
            # ---- phase 2: chunks 1-3 K/V projections via fp8 DoubleRow
            ps_k = {c: poolM.tile([128, 512], F32, tag="m512", name=f"kps{c}")
                    for c in (1, 2, 3)}
            ps_v = {c: poolO.tile([128, 512], F32, tag="o512", name=f"vps{c}")
                    for c in (1, 2)}
            for kp in range(KT // 2):
                sl = slice(2 * kp, 2 * kp + 2)
                for c in (1, 2, 3):
                    nc.tensor.matmul(ps_k[c], wk8_sb[:, sl, :],
                                     xT8_sb[:, sl, c * 512:(c + 1) * 512],
                                     start=(kp == 0), stop=(kp == KT // 2 - 1),
                                     perf_mode=DR)
                for c in (1, 2):
                    nc.tensor.matmul(ps_v[c], wv8_sb[:, sl, :],
                                     xT8_sb[:, sl, c * 512:(c + 1) * 512],
                                     start=(kp == 0), stop=(kp == KT // 2 - 1),
                                     perf_mode=DR)
            vp3 = poolO.tile([128, 512], F32, tag="o512", name="vps3")
            for kp in range(KT // 2):
                sl = slice(2 * kp, 2 * kp + 2)
                nc.tensor.matmul(vp3, wv8_sb[:, sl, :],
                                 xT8_sb[:, sl, 3 * 512:4 * 512],
                                 start=(kp == 0), stop=(kp == KT // 2 - 1),
                                 perf_mode=DR)
            ps_v[3] = vp3
            for c in (1, 2, 3):
                raw = evict_raw(ps_k[c], c)
                rope_store(raw, kT_sb[:, c * 512:(c + 1) * 512],
                           bk_sb[:, 0:1], c)
            for c in (1, 2, 3):
                v_evict(ps_v[c], c)

        def emit_q_proj(h):
            pss = {c: poolM.tile([128, 512], F32, tag="m512", name=f"qps{c}")
                   for c in (1, 2, 3)}
            for kp in range(KT // 2):
                sl = slice(2 * kp, 2 * kp + 2)
                for c in (1, 2, 3):
                    nc.tensor.matmul(pss[c],
                                     wq8_sb[:, sl, h * 128:(h + 1) * 128],
                                     xT8_sb[:, sl, c * 512:(c + 1) * 512],
                                     start=(kp == 0), stop=(kp == KT // 2 - 1),
                                     perf_mode=DR)
            for c in (1, 2, 3):
                raw = evict_raw(pss[c], c)
                rope_store(raw, qT_sb[h][:, c * 512:(c + 1) * 512],
                           bq_sb[:, h:h + 1], c)

        def emit_c_group(m, np_, ns=None, tail=False):
            if ns is None:
                ns = (2 * np_, 2 * np_ + 1)
            mc, mo = divmod(m, 4)
            pso = {n: poolM.tile([128, 512], F32, tag="m512", name=f"cpsum{n}")
                   for n in ns}
            for h in range(HPC):
                for n in ns:
                    nc.tensor.matmul(pso[n],
                                     outT_sb[mc][:, h, mo * 128:(mo + 1) * 128],
                                     wo_sb[:, h, n * 512:(n + 1) * 512],
                                     start=(h == 0), stop=(h == HPC - 1))
            for g in range(0, len(ns), 2):
                np2 = ns[g:g + 2]
                ob = osbp.tile([128, 2, 512], BF, tag="osb", name="osb")
                # evictions on DVE: keep ScalarE free for the attention exps
                for j, n in enumerate(np2):
                    nc.vector.tensor_copy(ob[:, j, :], pso[n])
                eng = nc.sync if tail else None
                if eng is None:
                    dma_out(out_d[m * 128:(m + 1) * 128,
                                  np2[0] * 512:(np2[-1] + 1) * 512], ob)
                else:
                    eng.dma_start(
                        out=out_d[m * 128:(m + 1) * 128,
                                  np2[0] * 512:(np2[-1] + 1) * 512],
                        in_=ob)

        def emit_b_pair(c, hp, fill_ms=(), tail_ms=None):
            npairs = c            # fp8 DoubleRow units of 2 kv tiles... 2*c pairs
            nt = 4 * c + 4
            hs = (2 * hp, 2 * hp + 1)
            cl, ch_ = c * 512, (c + 1) * 512
            out_ps = {h: poolO.tile([128, 512], F32, tag="o512",
                                    name=f"outps{h}") for h in hs}
            # both heads' exp-sum rows share one PSUM bank (partitions 0/32)
            sums_pair = poolO.tile([33, 512], F32, tag="o512", name="sums_pair")

            # units: 2c fp8 pairs (tiles 2u, 2u+1), then 4 bf16 diagonal tiles
            def emit_scores(u):
                es = {}
                if u < 2 * c:
                    for h in hs:
                        e8 = expp.tile([128, 2, 512], F8, tag="e8", name="e8")
                        for i in (0, 1):
                            t = 2 * u + i
                            s_ps = poolM.tile([128, 512], F32, tag="m512",
                                              name="s_ps")
                            nc.tensor.matmul(s_ps,
                                             kT_sb[:, t * 128:(t + 1) * 128],
                                             qT_sb[h][:, cl:ch_],
                                             start=True, stop=True)
                            nc.scalar.activation(e8[:, i, :], s_ps,
                                                 func=AF.Exp, scale=SCALE,
                                                 bias=negln2_sb[:, 0:1])
                        es[h] = e8
                else:
                    jb = u - 2 * c
                    t = 4 * c + jb
                    off = jb * 128
                    for h in hs:
                        s_ps = poolM.tile([128, 512], F32, tag="m512",
                                          name="s_ps")
                        nc.tensor.matmul(s_ps[:, off:],
                                         kT_sb[:, t * 128:(t + 1) * 128],
                                         qT_sb[h][:, cl + off:ch_],
                                         start=True, stop=True)
                        e = expb.tile([128, 512], BF, tag="exp", name="e")
                        nc.scalar.activation(e[:, off:], s_ps[:, off:],
                                             func=AF.Exp, scale=SCALE,
                                             bias=negln2_sb[:, 0:1])
                        nc.vector.tensor_mul(e[:, off:off + 128],
                                             e[:, off:off + 128], mask_sb)
                        es[h] = e
                return es

            def emit_consume(u, es):
                first = (u == 0)
                if u < 2 * c:
                    sl = slice(2 * u, 2 * u + 2)
                    # head 1 first (33-row write incl. zeroed rows 0-31), so
                    # head 0's start=True re-zeroes only row 0 afterwards
                    nc.tensor.matmul(sums_pair[0:33, :],
                                     ones8b_sb[:, :, 0:33], es[hs[1]],
                                     start=first, stop=False, perf_mode=DR)
                    nc.tensor.matmul(sums_pair[0:1, :],
                                     ones8_sb[:, :, 0:1], es[hs[0]],
                                     start=first, stop=False, perf_mode=DR)
                    for h in hs:
                        nc.tensor.matmul(out_ps[h], v8_sb[:, sl, :], es[h],
                                         start=first, stop=False,
                                         perf_mode=DR)
                else:
                    jb = u - 2 * c
                    t = 4 * c + jb
                    off = jb * 128
                    last = (jb == 3)
                    for i, h in enumerate(hs):
                        nc.tensor.matmul(sums_pair[32 * i:32 * i + 1, off:],
                                         ones_sb, es[h][:, off:],
                                         start=first, stop=last)
                    for h in hs:
                        nc.tensor.matmul(out_ps[h][:, off:], v_sb[:, t, :],
                                         es[h][:, off:],
                                         start=first, stop=last)

            # software pipeline: scores for u+1 issue before the ones/PV
            # consumers of u, so PE never waits on exp
            nu = 2 * c + 4
            prev = None
            for u in range(nu):
                es = emit_scores(u)
                if prev is not None:
                    emit_consume(u - 1, prev)
                prev = es
            emit_consume(nu - 1, prev)

            # evict accumulators to SBUF so the PSUM banks free quickly; the
            # reciprocal/broadcast/normalize chain runs lazily off-PSUM
            # (outT[c] is only needed by out-proj work a chunk boundary later).
            outU = {}
            for h in hs:
                u = outup.tile([128, 512], F32, tag="outU", name="outU")
                nc.vector.tensor_copy(u, out_ps[h])
                outU[h] = u
            # custom-DVE ops ignore the input AP's partition offset (read
            # partition 0): head 1's sums must first move to a partition-0
            # tile via a regular DVE copy (which does handle quadrant shifts)
            recips = {}
            for i, h in enumerate(hs):
                src = sums_pair[32 * i:32 * i + 1, :]
                if i > 0:
                    s1 = sumsp.tile([1, 512], F32, tag="s1", name="s1")
                    nc.vector.tensor_copy(s1, src)
                    src = s1
                recip = rcp.tile([1, 512], F32, tag="recip", name="recip")
                nc.vector.reciprocal_approx_fast(recip, src)
                recips[h] = recip
            if DEBUG and c == 0 and hp == 0:
                nc.sync.dma_start(out=dbg_u0, in_=outU[0])
                nc.sync.dma_start(out=dbg_r0, in_=recips[0])
                nc.sync.dma_start(out=dbg_rc, in_=recips[1])
                nc.sync.dma_start(out=dbg_u1, in_=outU[1])
            rbcs = {}
            for h in hs:
                rbc = rcp.tile([128, 512], F32, tag="rbc", name="rbc")
                nc.gpsimd.partition_broadcast(rbc, recips[h])
                rbcs[h] = rbc
            if DEBUG and c == 0 and hp == 0:
                nc.sync.dma_start(out=dbg_rb1, in_=rbcs[1])
            # fill the boundary chain latency with out-proj work of the
            # previous chunk (its outT rows are complete); on the final pair
            # these PE fills cover the recip/broadcast chain before the
            # slab-normalized tail out-proj below
            for m in fill_ms:
                for np_ in range(NCH // 2):
                    emit_c_group(m, np_)
            if tail_ms is None:
                for h in hs:
                    nc.vector.tensor_mul(outT_sb[c][:, h, :], outU[h], rbcs[h])
            else:
                # final pair: normalize per 128-col slab so each slab's
                # out-proj m-tile starts without waiting for the full chain
                for mo in range(4):
                    sl = slice(mo * 128, (mo + 1) * 128)
                    for h in hs:
                        nc.vector.tensor_mul(outT_sb[c][:, h, sl],
                                             outU[h][:, sl], rbcs[h][:, sl])
                    emit_c_group(tail_ms[mo], 0, ns=(0, 1, 2, 3), tail=True)

        # ---- interleave: attention chunk 0 needs only phase-1 (bf16)
        # results, so it runs while the fp8 slabs finish streaming in; the
        # DoubleRow projections follow.
        emit_b_pair(0, 0)
        emit_b_pair(0, 1)
        emit_kv_dr()
        for h in range(HPC):
            emit_q_proj(h)
        for c in range(1, NCH):
            for hp in range(HPC // 2):
                base = 4 * (c - 1) + 2 * hp
                last = (c == NCH - 1 and hp == HPC // 2 - 1)
                emit_b_pair(c, hp, fill_ms=(base, base + 1),
                            tail_ms=tuple(range(12, 16)) if last else None)
        if DEBUG:
            nc.sync.dma_start(out=dbg_q, in_=qT_sb[0])
            nc.sync.dma_start(out=dbg_k, in_=kT_sb)
            nc.sync.dma_start(out=dbg_v, in_=v_sb)
            nc.sync.dma_start(out=dbg_o0, in_=outT_sb[0])

    nc.compile()
    return nc


def _get_program():
    global _PROGRAM
    if _PROGRAM is None:
        _PROGRAM = _build_program()
    return _PROGRAM


def _host_tables():
    bf16 = ml_dtypes.bfloat16
    pos = np.arange(S, dtype=np.float32)[:, None]
    i = np.arange(DH // 2, dtype=np.float32)
    omega = np.exp((-2.0 * i / DH * np.log(np.float32(1_000_000.0))).astype(np.float32))
    ang = (pos * omega).astype(np.float32)
    sinT = np.ascontiguousarray(np.repeat(np.sin(ang), 2, axis=-1).T)
    cosT = np.ascontiguousarray(np.repeat(np.cos(ang), 2, axis=-1).T)
    # fold the rotate-half sign into the sin table: row 2i -> -sin, 2i+1 -> +sin
    rowsign = np.where(np.arange(DH) % 2 == 0, -1.0, 1.0).astype(np.float32)
    sinTs = sinT * rowsign[:, None]
    maskLT = np.triu(np.ones((128, 128), np.float32)).astype(bf16)
    return cosT.astype(bf16), sinTs.astype(bf16), maskLT


def _install_ntff_hook():
    """Optional: register the axon NTFF profiling hook (missing antenv.axon_hooks
    shim) so run_bass_kernel_spmd(trace=True) can capture HW exec time."""
    import types
    try:
        import antenv
        if 'antenv.axon_hooks' not in sys.modules:
            mod = types.ModuleType('antenv.axon_hooks')
            _hook = [None]
            mod.set_axon_ntff_profile_hook = lambda h: _hook.__setitem__(0, h)
            mod.get_axon_ntff_profile_hook = lambda: _hook[0]
            sys.modules['antenv.axon_hooks'] = mod
            antenv.axon_hooks = mod
        if '/root/.axon_site' not in sys.path:
            sys.path.insert(0, '/root/.axon_site')
        from trn_agent_boot.trn_boot import _ntff_profile_via_ctypes
        sys.modules['antenv.axon_hooks'].set_axon_ntff_profile_hook(
            _ntff_profile_via_ctypes('/opt/axon/libaxon_pjrt.so'))
        bass_utils.upload_artifacts = lambda tmpdir: tmpdir
        return True
    except Exception:
        return False


def kernel(x, wq, bq, wk, bk, wv, bv, wo, bo, masked=None, **_unused):
    global LAST_RESULTS
    bf16 = ml_dtypes.bfloat16
    f8 = ml_dtypes.float8_e4m3
    nc = _get_program()

    x = np.asarray(x, np.float32)
    wq = np.asarray(wq, np.float32)
    wk = np.asarray(wk, np.float32)
    wv = np.asarray(wv, np.float32)
    wo = np.asarray(wo, np.float32)
    bq = np.asarray(bq, np.float32)
    bk = np.asarray(bk, np.float32)
    bv = np.asarray(bv, np.float32)
    bo = np.asarray(bo, np.float32)

    cosT, sinTs, maskLT = _host_tables()

    def tileize(aT, cols):
        # [D, cols] -> [128, KT*cols] partition-major (matches SBUF layout)
        return np.ascontiguousarray(
            aT.reshape(KT, 128, cols).transpose(1, 0, 2).reshape(128, KT * cols))

    xT = [np.ascontiguousarray(x[b].T) for b in range(B)]
    xT8 = [tileize(np.asarray(t, f8), S) for t in xT]
    xTb = [tileize(np.asarray(t[:, :512], bf16), 512) for t in xT]
    in_maps = []
    for core in range(N_CORES):
        b, g = divmod(core, G)
        cs = slice(g * 512, (g + 1) * 512)          # q-channel / out-channel slice
        ks = slice(g * 128, (g + 1) * 128)          # kv-channel slice
        wqT = np.ascontiguousarray(wq[cs, :].T)
        wkT = np.ascontiguousarray(wk[ks, :].T)
        wvT = np.ascontiguousarray(wv[ks, :].T)
        in_maps.append({
            "xT8": xT8[b],
            "xTb": xTb[b],
            "wqTb": tileize(wqT.astype(bf16), 512),
            "wqT8": tileize(np.asarray(wqT * WS, f8), 512),
            "wkTb": tileize(wkT.astype(bf16), DH),
            "wkT8": tileize(np.asarray(wkT * WS, f8), DH),
            "wvTb": tileize(wvT.astype(bf16), DH),
            "wvT8": tileize(np.asarray(wvT * WS, f8), DH),
            "woT": np.ascontiguousarray(wo[:, cs].T).astype(bf16),
            "cosT": cosT,
            "sinTs": sinTs,
            "maskLT": maskLT,
            "bq": np.ascontiguousarray(bq[cs].reshape(HPC, DH).T),
            "bk": np.ascontiguousarray(bk[ks].reshape(DH, 1)),
            "bv": np.ascontiguousarray(bv[ks].reshape(DH, 1)),
        })

    trace = bool(TRACE)
    if trace:
        trace = _install_ntff_hook()
    res = bass_utils.run_bass_kernel_spmd(nc, in_maps,
                                          core_ids=list(range(N_CORES)),
                                          trace=trace)
    LAST_RESULTS = res

    out = np.zeros((B, S, D), np.float32)
    for core in range(N_CORES):
        b = core // G
        out[b] += np.asarray(res.results[core]["part"], np.float32)
    out += bo[None, None, :]
    return out


# revision 52
# speedup vs baseline: 1.2296x; 1.0584x over previous
"""Trainium2 Bass kernel for GQA attention (B=2, S=2048, D=2048, H=16, G=4 kv-heads,
DH=128) with interleaved RoPE (base 1e6) and causal mask.

Sharding: one (batch b, kv-group g) pair per NeuronCore -> 8 cores. Each core
computes its 4 q-heads against its single kv-head (Megatron-style column-split
of w_q/w_k/w_v, row-split of w_o) and produces a partial (S, D) output-projection
product in bf16; the host sums the 4 partials per batch and adds bo.

Mixed-precision dataflow (f32 PSUM accumulate everywhere):
  - Q/K/V projections: sequence chunk 0 (rows 0..511) in bf16; chunks 1-3 via
    fp8e4 DoubleRow matmuls (two dmodel k-tiles per pass, 2x PE throughput).
    fp8 weights are prescaled x32 on host (dodges e4m3 subnormals) and the
    PSUM eviction applies 1/32.
  - Interleaved RoPE in transposed layout via DVE stream_shuffle (partition
    pair swap) + sign-folded sin table; no PE permutation matmul.
  - Attention per (head, 512-col sq chunk): scoresT tiles [sk=128, sq<=512]
    bf16 on PE; exp on ScalarE with bias -ln2 (keeps e under fp8e4 max) ->
    off-diagonal kv-tile pairs stored fp8 [128,2,512], diagonal tiles bf16
    (masked via DVE). PV and the ones-row exp-sum matmuls run fp8 DoubleRow
    on pairs, bf16 on diagonal tiles, accumulating in shared PSUM.
  - Softmax denominators: reciprocal_approx_fast (51 ULP), partition
    broadcast on GpSimd, normalize-on-evict via DVE. (Custom DVE ops ignore
    the input AP partition offset, so head 1's sums first move to a
    partition-0 tile with a regular DVE copy.)
  - Output projection bf16 (errors there hit the final output unattenuated,
    so no fp8), PSUM evictions on DVE to keep ScalarE free for exps,
    partial streamed to DRAM as bf16 in 2-tile slabs.

Early causal rows (chunk 0) only ever touch bf16 q/k/v/e, so fp8 noise is
always attenuated by softmax averaging over >=512 keys.

Schedule: all inputs are host-pre-tiled to [128, KT*cols] so every DMA is a
contiguous 2D transfer (sub-2KB-run patterns measured ~4x slower); phase 1
(bf16 chunk-0 K/V/Q + ropes) starts off ~130KB of head-of-stream mini DMAs
while the fp8 slabs stream in; attention chunk 0 needs only phase-1 results
and covers the rest of the input stream; out-proj work for completed chunks
fills each pair's normalize-chain latency; the final pair normalizes per
128-col slab so each slab immediately unlocks its out-proj m-tile.
"""
import sys
import os

if '/opt/trn_rl_repo' not in sys.path:
    sys.path.insert(0, '/opt/trn_rl_repo')

import numpy as np
import ml_dtypes
import math

from contextlib import ExitStack

import concourse.bass as bass
import concourse.mybir as mybir
import concourse.tile as tile
from concourse import bacc
import concourse.bass_utils as bass_utils
from concourse.masks import make_identity

BF = mybir.dt.bfloat16
F32 = mybir.dt.float32
F8 = mybir.dt.float8e4
AF = mybir.ActivationFunctionType
ALU = mybir.AluOpType
DR = mybir.MatmulPerfMode.DoubleRow

B, S, D, H, G = 2, 2048, 2048, 16, 4
DH = 128
HPC = H // G          # q heads per core
KT = D // 128         # dmodel k-tiles
NCH = S // 512        # sq chunks
SCALE = float(1.0 / np.sqrt(DH))
NEGLN2 = float(-np.log(2.0))
WS = 32.0             # host-side fp8 weight prescale
N_CORES = 8
SWAP_MASK = [i ^ 1 for i in range(32)]   # partition pair swap for RoPE

TRACE = False          # set by test harness to capture an NTFF profile
DEBUG = False          # add intermediate-tensor outputs for stage-wise checks
LAST_RESULTS = None    # BassKernelResults of the most recent run (for test.py)

_PROGRAM = None


def _build_program():
    nc = bacc.Bacc("TRN2", target_bir_lowering=False, debug=False,
                   num_devices=N_CORES)

    def din(name, shape, dtype=BF):
        return nc.dram_tensor(name, shape, dtype, kind="ExternalInput").ap()

    # pre-tiled on host to [128, KT*cols] partition-major layout so every
    # input DMA is a plain 2D transfer with multi-KB contiguous runs
    xT8_d = din("xT8", [128, KT * S], F8)
    xTb_d = din("xTb", [128, KT * 512])
    wqb_d = din("wqTb", [128, KT * 512])
    wq8_d = din("wqT8", [128, KT * 512], F8)
    wkb_d = din("wkTb", [128, KT * DH])
    wk8_d = din("wkT8", [128, KT * DH], F8)
    wvb_d = din("wvTb", [128, KT * DH])
    wv8_d = din("wvT8", [128, KT * DH], F8)
    wo_d = din("woT", [512, D])
    cos_d = din("cosT", [DH, S])
    sin_d = din("sinTs", [DH, S])     # sign-folded sin table
    mask_d = din("maskLT", [DH, DH])
    bq_d = din("bq", [DH, HPC], F32)
    bk_d = din("bk", [DH, 1], F32)
    bv_d = din("bv", [DH, 1], F32)
    out_d = nc.dram_tensor("part", [S, D], BF, kind="ExternalOutput").ap()
    if DEBUG:
        dbg_q = nc.dram_tensor("dbg_q", [128, S], BF, kind="ExternalOutput").ap()
        dbg_k = nc.dram_tensor("dbg_k", [128, S], BF, kind="ExternalOutput").ap()
        dbg_v = nc.dram_tensor("dbg_v", [128, KT * DH], BF, kind="ExternalOutput").ap()
        dbg_o0 = nc.dram_tensor("dbg_o0", [128, HPC * 512], BF, kind="ExternalOutput").ap()
        dbg_u0 = nc.dram_tensor("dbg_u0", [128, 512], F32, kind="ExternalOutput").ap()
        dbg_r0 = nc.dram_tensor("dbg_r0", [1, 512], F32, kind="ExternalOutput").ap()
        dbg_rc = nc.dram_tensor("dbg_rc", [1, 512], F32, kind="ExternalOutput").ap()
        dbg_rb1 = nc.dram_tensor("dbg_rb1", [128, 512], F32, kind="ExternalOutput").ap()
        dbg_u1 = nc.dram_tensor("dbg_u1", [128, 512], F32, kind="ExternalOutput").ap()

    with tile.TileContext(nc) as tc, ExitStack() as ctx:
        consts = ctx.enter_context(tc.tile_pool(name="consts", bufs=1))
        # Two global PSUM pools (no phase boundaries): poolM rotates among
        # k-proj/q-proj/vtrans/scores/out-proj psums, poolO among v-proj and
        # the attention out/sum accumulators.
        poolM = ctx.enter_context(tc.tile_pool(name="poolM", bufs=5, space="PSUM"))
        poolO = ctx.enter_context(tc.tile_pool(name="poolO", bufs=3, space="PSUM"))
        rawp = ctx.enter_context(tc.tile_pool(name="rawp", bufs=2))
        tmpp = ctx.enter_context(tc.tile_pool(name="tmpp", bufs=3))
        expp = ctx.enter_context(tc.tile_pool(name="expp", bufs=6))
        expb = ctx.enter_context(tc.tile_pool(name="expb", bufs=6))
        rcp = ctx.enter_context(tc.tile_pool(name="rcp", bufs=3))
        osbp = ctx.enter_context(tc.tile_pool(name="osbp", bufs=2))
        outup = ctx.enter_context(tc.tile_pool(name="outup", bufs=4))
        sumsp = ctx.enter_context(tc.tile_pool(name="sumsp", bufs=3))

        # persistent SBUF tensors
        wqb_sb = consts.tile([128, KT, 512], BF, tag="wqb")
        wq8_sb = consts.tile([128, KT, 512], F8, tag="wq8")
        wkb_sb = consts.tile([128, KT, DH], BF, tag="wkb")
        wk8_sb = consts.tile([128, KT, DH], F8, tag="wk8")
        wvb_sb = consts.tile([128, KT, DH], BF, tag="wvb")
        wv8_sb = consts.tile([128, KT, DH], F8, tag="wv8")
        wo_sb = consts.tile([128, HPC, D], BF, tag="wo")
        mask_sb = consts.tile([128, 128], BF, tag="mask")
        bq_sb = consts.tile([128, HPC], F32, tag="bq")
        bk_sb = consts.tile([128, 1], F32, tag="bk")
        bv_sb = consts.tile([128, 1], F32, tag="bv")
        ones_sb = consts.tile([128, 1], BF, tag="ones")
        # dual-fp8 LDWEIGHTS needs the k-pair dim step to be a multiple of 16,
        # and DoubleRow matmuls must write PSUM starting at partition 0: head 1
        # uses a zero-padded ones column at index 32 so its sums land on
        # partition 32 of a partition-0-based [33, 512] output.
        ones8_sb = consts.tile([128, 2, 16], F8, tag="ones8")
        ones8b_sb = consts.tile([128, 2, 48], F8, tag="ones8b")
        negln2_sb = consts.tile([128, 1], F32, tag="negln2")
        qT_sb = [consts.tile([128, S], BF, tag=f"qT{h}", name=f"qT{h}")
                 for h in range(HPC)]
        kT_sb = consts.tile([128, S], BF, tag="kT")
        v_sb = consts.tile([128, KT, DH], BF, tag="v")
        v8_sb = consts.tile([128, KT, DH], F8, tag="v8")
        outT_sb = [consts.tile([128, HPC, 512], BF, tag=f"outT{c}", name=f"outT{c}")
                   for c in range(NCH)]
        xT8_sb = consts.tile([128, KT, S], F8, tag="xT8")
        xTb_sb = consts.tile([128, KT, 512], BF, tag="xTb")
        cos_sb = consts.tile([128, S], BF, tag="cos")
        sin_sb = consts.tile([128, S], BF, tag="sin")
        ident_sb = consts.tile([128, 128], F32, tag="ident")

        # ---- input DMAs: consumption order, issue spread over 3 engine
        # sequencers (each dma_start costs ~0.6us of descriptor-gen on its
        # triggering sequencer).
        _dma_engines = [nc.sync, nc.gpsimd, nc.scalar]
        _dma_i = [0]

        def dma_in(out, in_):
            eng = _dma_engines[_dma_i[0] % len(_dma_engines)]
            _dma_i[0] += 1
            eng.dma_start(out=out, in_=in_)

        def ld(eng, sb, kk0, kk1, dram, cols):
            eng.dma_start(
                out=sb[:, kk0:kk1, :].rearrange("p t c -> p (t c)"),
                in_=dram[:, kk0 * cols:kk1 * cols])

        # Explicit queue plan. sync+gpsimd carry the PE-critical stream in
        # consumption order; the scalar sequencer (idle until evictions start)
        # takes the tiny tensors and the fp8 weights. ~13.5MB of input means
        # ~35us of HBM time: phase 1 + attention chunk 0 must cover it.
        # head-of-stream minis: exactly what the first K/V c0 matmuls need,
        # so the PE starts ~9us instead of ~16us
        ld(nc.sync, wkb_sb, 0, 2, wkb_d, DH)
        ld(nc.gpsimd, wvb_sb, 0, 2, wvb_d, DH)
        ld(nc.sync, xTb_sb, 0, 1, xTb_d, 512)
        ld(nc.gpsimd, xTb_sb, 1, 3, xTb_d, 512)
        ld(nc.sync, xTb_sb, 3, 4, xTb_d, 512)
        ld(nc.sync, wkb_sb, 2, KT, wkb_d, DH)
        ld(nc.gpsimd, wvb_sb, 2, KT, wvb_d, DH)
        ld(nc.sync, xTb_sb, 4, 10, xTb_d, 512)
        ld(nc.gpsimd, xTb_sb, 10, KT, xTb_d, 512)
        nc.scalar.dma_start(out=bk_sb, in_=bk_d)
        nc.scalar.dma_start(out=bv_sb, in_=bv_d)
        nc.scalar.dma_start(out=bq_sb, in_=bq_d)
        nc.scalar.dma_start(out=mask_sb, in_=mask_d)
        nc.scalar.dma_start(out=cos_sb, in_=cos_d)
        nc.scalar.dma_start(out=sin_sb, in_=sin_d)
        ld(nc.sync, wqb_sb, 0, 8, wqb_d, 512)
        ld(nc.gpsimd, wqb_sb, 8, KT, wqb_d, 512)
        ld(nc.scalar, wk8_sb, 0, KT, wk8_d, DH)
        ld(nc.scalar, wv8_sb, 0, KT, wv8_d, DH)
        for qq in range(4):
            eng = (nc.sync, nc.gpsimd)[qq % 2]
            ld(eng, xT8_sb, qq * 4, qq * 4 + 4, xT8_d, S)
        ld(nc.scalar, wq8_sb, 0, KT, wq8_d, 512)
        for h in range(HPC):
            eng = (nc.sync, nc.gpsimd)[h % 2]
            eng.dma_start(out=wo_sb[:, h, :], in_=wo_d[h * 128:(h + 1) * 128, :])

        # output DMAs round-robin over sync+gpsimd (scalar is ACT-busy)
        _dmo_i = [0]

        def dma_out(out, in_):
            eng = (nc.sync, nc.gpsimd)[_dmo_i[0] % 2]
            _dmo_i[0] += 1
            eng.dma_start(out=out, in_=in_)
        nc.vector.memset(ones_sb, 1.0)
        nc.vector.memset(ones8_sb, 1.0)
        nc.vector.memset(ones8b_sb, 0.0)
        nc.vector.memset(ones8b_sb[:, :, 32:33], 1.0)
        nc.vector.memset(negln2_sb, NEGLN2)
        make_identity(nc, ident_sb)

        def rope_store(raw, dst, bias_ap, c):
            # interleaved RoPE: rot = pair-swap(raw) with sign folded into sin
            sh = tmpp.tile([128, 512], BF, tag="sh", name="sh")
            nc.vector.stream_shuffle(sh, raw, SWAP_MASK)
            t1 = tmpp.tile([128, 512], BF, tag="t1", name="t1")
            nc.vector.tensor_mul(t1, raw, cos_sb[:, c * 512:(c + 1) * 512])
            t2 = tmpp.tile([128, 512], BF, tag="t2", name="t2")
            nc.vector.tensor_mul(t2, sh, sin_sb[:, c * 512:(c + 1) * 512])
            # dst = (t2 + bias) + t1
            nc.vector.scalar_tensor_tensor(dst, t2, bias_ap, t1,
                                           op0=ALU.add, op1=ALU.add)

        def evict_raw(ps, c, dtype=BF, bias=None):
            raw = rawp.tile([128, 512], dtype, tag="raw", name="raw")
            scale = 1.0 if c == 0 else 1.0 / WS
            if bias is None:
                if c == 0:
                    nc.scalar.copy(raw, ps)
                else:
                    nc.scalar.activation(raw, ps, func=AF.Identity, scale=scale)
            else:
                nc.scalar.activation(raw, ps, func=AF.Identity, bias=bias,
                                     scale=scale)
            return raw

        def v_evict(ps, c):
            vraw = evict_raw(ps, c, dtype=F32, bias=bv_sb[:, 0:1])
            for j in range(4):
                t = c * 4 + j
                tp = poolM.tile([128, 128], F32, tag="m512", name="vtps")
                nc.tensor.transpose(tp, vraw[:, j * 128:(j + 1) * 128],
                                    ident_sb)
                nc.vector.tensor_copy(v_sb[:, t, :], tp)
                nc.vector.tensor_copy(v8_sb[:, t, :], tp)

        # ---- phase 1 (all bf16, needs only chunk-0 operands): K/V/Q chunk-0
        # projections + ropes, so the PE ramps while the fp8 slabs stream in.
        ps_k0 = poolM.tile([128, 512], F32, tag="m512", name="kps0")
        ps_v0 = poolO.tile([128, 512], F32, tag="o512", name="vps0")
        for kk in range(KT):
            nc.tensor.matmul(ps_k0, wkb_sb[:, kk, :], xTb_sb[:, kk, :],
                             start=(kk == 0), stop=(kk == KT - 1))
            nc.tensor.matmul(ps_v0, wvb_sb[:, kk, :], xTb_sb[:, kk, :],
                             start=(kk == 0), stop=(kk == KT - 1))
        raw = evict_raw(ps_k0, 0)
        rope_store(raw, kT_sb[:, 0:512], bk_sb[:, 0:1], 0)
        v_evict(ps_v0, 0)
        for h in range(HPC):
            psq = poolM.tile([128, 512], F32, tag="m512", name=f"qc0ps{h}")
            for kk in range(KT):
                nc.tensor.matmul(psq, wqb_sb[:, kk, h * 128:(h + 1) * 128],
                                 xTb_sb[:, kk, :],
                                 start=(kk == 0), stop=(kk == KT - 1))
            raw = evict_raw(psq, 0)
            rope_store(raw, qT_sb[h][:, 0:512], bq_sb[:, h:h + 1], 0)

        def emit_kv_dr():
            # ---- phase 2: chunks 1-3 K/V projections via fp8 DoubleRow
            ps_k = {c: poolM.tile([128, 512], F32, tag="m512", name=f"kps{c}")
                    for c in (1, 2, 3)}
            ps_v = {c: poolO.tile([128, 512], F32, tag="o512", name=f"vps{c}")
                    for c in (1, 2)}
            for kp in range(KT // 2):
                sl = slice(2 * kp, 2 * kp + 2)
                for c in (1, 2, 3):
                    nc.tensor.matmul(ps_k[c], wk8_sb[:, sl, :],
                                     xT8_sb[:, sl, c * 512:(c + 1) * 512],
                                     start=(kp == 0), stop=(kp == KT // 2 - 1),
                                     perf_mode=DR)
                for c in (1, 2):
                    nc.tensor.matmul(ps_v[c], wv8_sb[:, sl, :],
                                     xT8_sb[:, sl, c * 512:(c + 1) * 512],
                                     start=(kp == 0), stop=(kp == KT // 2 - 1),
                                     perf_mode=DR)
            vp3 = poolO.tile([128, 512], F32, tag="o512", name="vps3")
            for kp in range(KT // 2):
                sl = slice(2 * kp, 2 * kp + 2)
                nc.tensor.matmul(vp3, wv8_sb[:, sl, :],
                                 xT8_sb[:, sl, 3 * 512:4 * 512],
                                 start=(kp == 0), stop=(kp == KT // 2 - 1),
                                 perf_mode=DR)
            ps_v[3] = vp3
            for c in (1, 2, 3):
                raw = evict_raw(ps_k[c], c)
                rope_store(raw, kT_sb[:, c * 512:(c + 1) * 512],
                           bk_sb[:, 0:1], c)
            for c in (1, 2, 3):
                v_evict(ps_v[c], c)

        def emit_q_proj(h):
            pss = {c: poolM.tile([128, 512], F32, tag="m512", name=f"qps{c}")
                   for c in (1, 2, 3)}
            for kp in range(KT // 2):
                sl = slice(2 * kp, 2 * kp + 2)
                for c in (1, 2, 3):
                    nc.tensor.matmul(pss[c],
                                     wq8_sb[:, sl, h * 128:(h + 1) * 128],
                                     xT8_sb[:, sl, c * 512:(c + 1) * 512],
                                     start=(kp == 0), stop=(kp == KT // 2 - 1),
                                     perf_mode=DR)
            for c in (1, 2, 3):
                raw = evict_raw(pss[c], c)
                rope_store(raw, qT_sb[h][:, c * 512:(c + 1) * 512],
                           bq_sb[:, h:h + 1], c)

        def emit_c_group(m, np_, ns=None, tail=False):
            if ns is None:
                ns = (2 * np_, 2 * np_ + 1)
            mc, mo = divmod(m, 4)
            pso = {n: poolM.tile([128, 512], F32, tag="m512", name=f"cpsum{n}")
                   for n in ns}
            for h in range(HPC):
                for n in ns:
                    nc.tensor.matmul(pso[n],
                                     outT_sb[mc][:, h, mo * 128:(mo + 1) * 128],
                                     wo_sb[:, h, n * 512:(n + 1) * 512],
                                     start=(h == 0), stop=(h == HPC - 1))
            for g in range(0, len(ns), 2):
                np2 = ns[g:g + 2]
                ob = osbp.tile([128, 2, 512], BF, tag="osb", name="osb")
                # evictions on DVE: keep ScalarE free for the attention exps
                for j, n in enumerate(np2):
                    nc.vector.tensor_copy(ob[:, j, :], pso[n])
                eng = nc.sync if tail else None
                if eng is None:
                    dma_out(out_d[m * 128:(m + 1) * 128,
                                  np2[0] * 512:(np2[-1] + 1) * 512], ob)
                else:
                    eng.dma_start(
                        out=out_d[m * 128:(m + 1) * 128,
                                  np2[0] * 512:(np2[-1] + 1) * 512],
                        in_=ob)

        def emit_b_pair(c, hp, fill_ms=(), tail_ms=None):
            npairs = c            # fp8 DoubleRow units of 2 kv tiles... 2*c pairs
            nt = 4 * c + 4
            hs = (2 * hp, 2 * hp + 1)
            cl, ch_ = c * 512, (c + 1) * 512
            out_ps = {h: poolO.tile([128, 512], F32, tag="o512",
                                    name=f"outps{h}") for h in hs}
            # both heads' exp-sum rows share one PSUM bank (partitions 0/32)
            sums_pair = poolO.tile([33, 512], F32, tag="o512", name="sums_pair")

            # units: 2c fp8 pairs (tiles 2u, 2u+1), then 4 bf16 diagonal tiles
            def emit_scores(u):
                es = {}
                if u < 2 * c:
                    for h in hs:
                        e8 = expp.tile([128, 2, 512], F8, tag="e8", name="e8")
                        for i in (0, 1):
                            t = 2 * u + i
                            s_ps = poolM.tile([128, 512], F32, tag="m512",
                                              name="s_ps")
                            nc.tensor.matmul(s_ps,
                                             kT_sb[:, t * 128:(t + 1) * 128],
                                             qT_sb[h][:, cl:ch_],
                                             start=True, stop=True)
                            nc.scalar.activation(e8[:, i, :], s_ps,
                                                 func=AF.Exp, scale=SCALE,
                                                 bias=negln2_sb[:, 0:1])
                        es[h] = e8
                else:
                    jb = u - 2 * c
                    t = 4 * c + jb
                    off = jb * 128
                    for h in hs:
                        s_ps = poolM.tile([128, 512], F32, tag="m512",
                                          name="s_ps")
                        nc.tensor.matmul(s_ps[:, off:],
                                         kT_sb[:, t * 128:(t + 1) * 128],
                                         qT_sb[h][:, cl + off:ch_],
                                         start=True, stop=True)
                        e = expb.tile([128, 512], BF, tag="exp", name="e")
                        nc.scalar.activation(e[:, off:], s_ps[:, off:],
                                             func=AF.Exp, scale=SCALE,
                                             bias=negln2_sb[:, 0:1])
                        nc.vector.tensor_mul(e[:, off:off + 128],
                                             e[:, off:off + 128], mask_sb)
                        es[h] = e
                return es

            def emit_consume(u, es):
                first = (u == 0)
                if u < 2 * c:
                    sl = slice(2 * u, 2 * u + 2)
                    # head 1 first (33-row write incl. zeroed rows 0-31), so
                    # head 0's start=True re-zeroes only row 0 afterwards
                    nc.tensor.matmul(sums_pair[0:33, :],
                                     ones8b_sb[:, :, 0:33], es[hs[1]],
                                     start=first, stop=False, perf_mode=DR)
                    nc.tensor.matmul(sums_pair[0:1, :],
                                     ones8_sb[:, :, 0:1], es[hs[0]],
                                     start=first, stop=False, perf_mode=DR)
                    for h in hs:
                        nc.tensor.matmul(out_ps[h], v8_sb[:, sl, :], es[h],
                                         start=first, stop=False,
                                         perf_mode=DR)
                else:
                    jb = u - 2 * c
                    t = 4 * c + jb
                    off = jb * 128
                    last = (jb == 3)
                    for i, h in enumerate(hs):
                        nc.tensor.matmul(sums_pair[32 * i:32 * i + 1, off:],
                                         ones_sb, es[h][:, off:],
                                         start=first, stop=last)
                    for h in hs:
                        nc.tensor.matmul(out_ps[h][:, off:], v_sb[:, t, :],
                                         es[h][:, off:],
                                         start=first, stop=last)

            # single-PSUM out-proj fill jobs from the previous chunk,
            # interleaved into the unit loop: 4 in-flight score psums + 1
            # fill psum exactly fits poolM, and the fill matmuls occupy the
            # PE during each unit's ScalarE exp latency
            jobs = [(m, n) for m in fill_ms for n in range(NCH)]
            job_slab = [None]

            def emit_fill_job():
                if not jobs:
                    return
                m, n = jobs.pop(0)
                mc, mo = divmod(m, 4)
                ps = poolM.tile([128, 512], F32, tag="m512", name="fill_ps")
                for h in range(HPC):
                    nc.tensor.matmul(ps,
                                     outT_sb[mc][:, h, mo * 128:(mo + 1) * 128],
                                     wo_sb[:, h, n * 512:(n + 1) * 512],
                                     start=(h == 0), stop=(h == HPC - 1))
                if n % 2 == 0:
                    job_slab[0] = osbp.tile([128, 2, 512], BF, tag="osb",
                                            name="osb")
                nc.vector.tensor_copy(job_slab[0][:, n % 2, :], ps)
                if n % 2 == 1:
                    dma_out(out_d[m * 128:(m + 1) * 128,
                                  (n - 1) * 512:(n + 1) * 512], job_slab[0])

            # software pipeline: scores for u+1 issue before the ones/PV
            # consumers of u, so PE never waits on exp
            nu = 2 * c + 4
            prev = None
            for u in range(nu):
                es = emit_scores(u)
                if prev is not None:
                    emit_consume(u - 1, prev)
                    emit_fill_job()
                prev = es
            emit_consume(nu - 1, prev)

            # evict accumulators to SBUF so the PSUM banks free quickly; the
            # reciprocal/broadcast/normalize chain runs lazily off-PSUM
            # (outT[c] is only needed by out-proj work a chunk boundary later).
            outU = {}
            for h in hs:
                u = outup.tile([128, 512], F32, tag="outU", name="outU")
                nc.vector.tensor_copy(u, out_ps[h])
                outU[h] = u
            # custom-DVE ops ignore the input AP's partition offset (read
            # partition 0): head 1's sums must first move to a partition-0
            # tile via a regular DVE copy (which does handle quadrant shifts)
            recips = {}
            for i, h in enumerate(hs):
                src = sums_pair[32 * i:32 * i + 1, :]
                if i > 0:
                    s1 = sumsp.tile([1, 512], F32, tag="s1", name="s1")
                    nc.vector.tensor_copy(s1, src)
                    src = s1
                recip = rcp.tile([1, 512], F32, tag="recip", name="recip")
                nc.vector.reciprocal_approx_fast(recip, src)
                recips[h] = recip
            if DEBUG and c == 0 and hp == 0:
                nc.sync.dma_start(out=dbg_u0, in_=outU[0])
                nc.sync.dma_start(out=dbg_r0, in_=recips[0])
                nc.sync.dma_start(out=dbg_rc, in_=recips[1])
                nc.sync.dma_start(out=dbg_u1, in_=outU[1])
            rbcs = {}
            for h in hs:
                rbc = rcp.tile([128, 512], F32, tag="rbc", name="rbc")
                nc.gpsimd.partition_broadcast(rbc, recips[h])
                rbcs[h] = rbc
            if DEBUG and c == 0 and hp == 0:
                nc.sync.dma_start(out=dbg_rb1, in_=rbcs[1])
            # leftover fill jobs cover the recip/broadcast chain latency
            while jobs:
                emit_fill_job()
            if tail_ms is None:
                for h in hs:
                    nc.vector.tensor_mul(outT_sb[c][:, h, :], outU[h], rbcs[h])
            else:
                # final pair: normalize per 128-col slab so each slab's
                # out-proj m-tile starts without waiting for the full chain
                for mo in range(4):
                    sl = slice(mo * 128, (mo + 1) * 128)
                    for h in hs:
                        nc.vector.tensor_mul(outT_sb[c][:, h, sl],
                                             outU[h][:, sl], rbcs[h][:, sl])
                    emit_c_group(tail_ms[mo], 0, ns=(0, 1, 2, 3), tail=True)

        # ---- interleave: attention chunk 0 needs only phase-1 (bf16)
        # results, so it runs while the fp8 slabs finish streaming in; the
        # DoubleRow projections follow.
        emit_b_pair(0, 0)
        emit_b_pair(0, 1)
        emit_kv_dr()
        for h in range(HPC):
            emit_q_proj(h)
        for c in range(1, NCH):
            for hp in range(HPC // 2):
                base = 4 * (c - 1) + 2 * hp
                last = (c == NCH - 1 and hp == HPC // 2 - 1)
                emit_b_pair(c, hp, fill_ms=(base, base + 1),
                            tail_ms=tuple(range(12, 16)) if last else None)
        if DEBUG:
            nc.sync.dma_start(out=dbg_q, in_=qT_sb[0])
            nc.sync.dma_start(out=dbg_k, in_=kT_sb)
            nc.sync.dma_start(out=dbg_v, in_=v_sb)
            nc.sync.dma_start(out=dbg_o0, in_=outT_sb[0])

    nc.compile()
    return nc


def _get_program():
    global _PROGRAM
    if _PROGRAM is None:
        _PROGRAM = _build_program()
    return _PROGRAM


def _host_tables():
    bf16 = ml_dtypes.bfloat16
    pos = np.arange(S, dtype=np.float32)[:, None]
    i = np.arange(DH // 2, dtype=np.float32)
    omega = np.exp((-2.0 * i / DH * np.log(np.float32(1_000_000.0))).astype(np.float32))
    ang = (pos * omega).astype(np.float32)
    sinT = np.ascontiguousarray(np.repeat(np.sin(ang), 2, axis=-1).T)
    cosT = np.ascontiguousarray(np.repeat(np.cos(ang), 2, axis=-1).T)
    # fold the rotate-half sign into the sin table: row 2i -> -sin, 2i+1 -> +sin
    rowsign = np.where(np.arange(DH) % 2 == 0, -1.0, 1.0).astype(np.float32)
    sinTs = sinT * rowsign[:, None]
    maskLT = np.triu(np.ones((128, 128), np.float32)).astype(bf16)
    return cosT.astype(bf16), sinTs.astype(bf16), maskLT


def _install_ntff_hook():
    """Optional: register the axon NTFF profiling hook (missing antenv.axon_hooks
    shim) so run_bass_kernel_spmd(trace=True) can capture HW exec time."""
    import types
    try:
        import antenv
        if 'antenv.axon_hooks' not in sys.modules:
            mod = types.ModuleType('antenv.axon_hooks')
            _hook = [None]
            mod.set_axon_ntff_profile_hook = lambda h: _hook.__setitem__(0, h)
            mod.get_axon_ntff_profile_hook = lambda: _hook[0]
            sys.modules['antenv.axon_hooks'] = mod
            antenv.axon_hooks = mod
        if '/root/.axon_site' not in sys.path:
            sys.path.insert(0, '/root/.axon_site')
        from trn_agent_boot.trn_boot import _ntff_profile_via_ctypes
        sys.modules['antenv.axon_hooks'].set_axon_ntff_profile_hook(
            _ntff_profile_via_ctypes('/opt/axon/libaxon_pjrt.so'))
        bass_utils.upload_artifacts = lambda tmpdir: tmpdir
        return True
    except Exception:
        return False


def kernel(x, wq, bq, wk, bk, wv, bv, wo, bo, masked=None, **_unused):
    global LAST_RESULTS
    bf16 = ml_dtypes.bfloat16
    f8 = ml_dtypes.float8_e4m3
    nc = _get_program()

    x = np.asarray(x, np.float32)
    wq = np.asarray(wq, np.float32)
    wk = np.asarray(wk, np.float32)
    wv = np.asarray(wv, np.float32)
    wo = np.asarray(wo, np.float32)
    bq = np.asarray(bq, np.float32)
    bk = np.asarray(bk, np.float32)
    bv = np.asarray(bv, np.float32)
    bo = np.asarray(bo, np.float32)

    cosT, sinTs, maskLT = _host_tables()

    def tileize(aT, cols):
        # [D, cols] -> [128, KT*cols] partition-major (matches SBUF layout)
        return np.ascontiguousarray(
            aT.reshape(KT, 128, cols).transpose(1, 0, 2).reshape(128, KT * cols))

    xT = [np.ascontiguousarray(x[b].T) for b in range(B)]
    xT8 = [tileize(np.asarray(t, f8), S) for t in xT]
    xTb = [tileize(np.asarray(t[:, :512], bf16), 512) for t in xT]
    in_maps = []
    for core in range(N_CORES):
        b, g = divmod(core, G)
        cs = slice(g * 512, (g + 1) * 512)          # q-channel / out-channel slice
        ks = slice(g * 128, (g + 1) * 128)          # kv-channel slice
        wqT = np.ascontiguousarray(wq[cs, :].T)
        wkT = np.ascontiguousarray(wk[ks, :].T)
        wvT = np.ascontiguousarray(wv[ks, :].T)
        in_maps.append({
            "xT8": xT8[b],
            "xTb": xTb[b],
            "wqTb": tileize(wqT.astype(bf16), 512),
            "wqT8": tileize(np.asarray(wqT * WS, f8), 512),
            "wkTb": tileize(wkT.astype(bf16), DH),
            "wkT8": tileize(np.asarray(wkT * WS, f8), DH),
            "wvTb": tileize(wvT.astype(bf16), DH),
            "wvT8": tileize(np.asarray(wvT * WS, f8), DH),
            "woT": np.ascontiguousarray(wo[:, cs].T).astype(bf16),
            "cosT": cosT,
            "sinTs": sinTs,
            "maskLT": maskLT,
            "bq": np.ascontiguousarray(bq[cs].reshape(HPC, DH).T),
            "bk": np.ascontiguousarray(bk[ks].reshape(DH, 1)),
            "bv": np.ascontiguousarray(bv[ks].reshape(DH, 1)),
        })

    trace = bool(TRACE)
    if trace:
        trace = _install_ntff_hook()
    res = bass_utils.run_bass_kernel_spmd(nc, in_maps,
                                          core_ids=list(range(N_CORES)),
                                          trace=trace)
    LAST_RESULTS = res

    out = np.zeros((B, S, D), np.float32)
    for core in range(N_CORES):
        b = core // G
        out[b] += np.asarray(res.results[core]["part"], np.float32)
    out += bo[None, None, :]
    return out
